# revision 8
# baseline (speedup 1.0000x reference)
"""DiagonalLSTM Trainium2 kernel.

Reference computation (per batch element b):
  xs = skew(x)                               # (Cin, H, 2W-1), row r shifted right by r
  z_is = w_is @ xs + b_is                    # 1x1 conv -> 4*HID channels
  for t in 0..2W-2:                          # sequential scan over skewed width
      hs[o, r] = wss[o,c,0] h[c,r-1] + wss[o,c,1] h[c,r] + b_ss[o]
      z = z_is[:, :, t] + hs
      i, f, o_, g = sig, sig, sig, tanh of the 4 gate quarters
      c = f*c + i*g ; h = o_*tanh(c)
  out = unskew(h history)

Sharding: data-parallel over batch B=8 across the 8 NeuronCores (the t-scan is
inherently sequential; each core runs its own batch element's full scan).

Per-core layout (128 partitions = channels), gate order [f, i, g, o]:
 - gates-on-partitions: per step the gates live in THREE psum banks -- f
   (128x64), i|g (128x128), o (128x64).
 - g is computed VIA SIGMOID: tanh(z) = 2*sigmoid(2z) - 1, the factor 2 folded
   into the g columns of all weights host-side.
 - Measured critical chain per step (HW trace, 2072ns at baseline):
     MM f-taps -> sig_f -> sig_ig -> t1 -> c -> TANH -> h-write
   Optimizations in this version, all aimed at that chain:
   * z_is(t+1) matmuls are DELAYED (dep on TANH(t-1)) so the PE streams them
     right up to the moment h(t-1) lands -> the first rec tap issues
     back-to-back instead of paying the isolated-matmul pipe-fill (~210ns).
   * an ACT dummy op after each TANH keeps the scalar engine's pipeline busy
     through the MM segment so sig_f starts back-to-back (saves the ~130-cycle
     restart bubble: 313ns -> ~200ns).
   * sig_o carries a dep on t1 so it issues late and TANH then runs
     back-to-back behind it on the ACT queue (314ns -> ~200ns).
   * sigmoid outputs are fp16 -> t1 runs in the DVE 2x mode.
   * t1 runs on the (otherwise idle) GPSIMD engine, shortening the VE serial
     segment to t2 -> c.
 - x is pre-skewed and pre-cast to fp16 ON THE HOST, t-major (zero padded);
   the step-t rhs is the contiguous slice xs[:, t*64:(t+1)*64].
 - h is written as fp16 into a (128 x 66) rhs buffer with cols 0:2 always 0;
   tap0 (h[r-1]) = cols 1:65 and tap1 (h[r]) = cols 2:66.
 - h history is stored fp32 directly in unskewed layout hist[c, r*64+w] via a
   stride-63 write of the in-band rows; the output DMA is chunked by row
   groups overlapping the scan tail.
 - zero-bias fast path: every per-step op covers only the ACTIVE row window
   [max(0,t-63) & ~1, min(t,63)].
"""

import sys

if "/opt/trn_rl_repo" not in sys.path:
    sys.path.insert(0, "/opt/trn_rl_repo")

import numpy as np

N_CORES = 8
HID = 128
CIN = 128
H = 64
W = 64
T = 2 * W - 1  # 127
LOOKAHEAD = 1
RCHUNK = 16  # epilogue row-chunk size

# --- scan-chain tunables ---
ACT_DUMW = 256  # width of the ACT keep-warm dummy (0 disables)
Z_DELAY = True  # delay z_is(t+1) MMs behind TANH(t-1) to fill the PE tail
GP_T2 = True    # compute t2 (plain TT mult) on gpsimd instead of vector
GP_HIST = True  # write the h history on gpsimd instead of vector
SIG_FP16 = True  # sigmoid outputs in fp16
SIGO_DELAY = True  # delay sig_o behind t1 so TANH runs back-to-back

_PROGRAM_CACHE = {}


def _build_program(use_bias: bool):
    import concourse.bacc as bacc
    import concourse.tile as tile
    from concourse import mybir

    fp32 = mybir.dt.float32
    fp16 = mybir.dt.float16
    AFT = mybir.ActivationFunctionType
    ALU = mybir.AluOpType

    nc = bacc.Bacc("TRN2", debug=False, num_devices=N_CORES)
    xs_d = nc.dram_tensor("xs", [CIN, T * 64], fp16, kind="ExternalInput")
    wis_d = nc.dram_tensor("wis", [CIN, 4 * HID], fp16, kind="ExternalInput")
    wss0_d = nc.dram_tensor("wss0", [HID, 4 * HID], fp16, kind="ExternalInput")
    wss1_d = nc.dram_tensor("wss1", [HID, 4 * HID], fp16, kind="ExternalInput")
    bias_d = nc.dram_tensor("bias", [HID, 4], fp32, kind="ExternalInput")
    out_d = nc.dram_tensor("out", [HID, H * W], fp32, kind="ExternalOutput")

    sig_dt = fp16 if SIG_FP16 else fp32

    with tile.TileContext(nc) as tc:
        with (
            tc.tile_pool(name="persist", bufs=1) as pp,
            tc.tile_pool(name="gates", bufs=3) as gp,
            tc.tile_pool(name="psf", bufs=2, space="PSUM") as psf,
            tc.tile_pool(name="psc", bufs=1, space="PSUM") as psc,
            tc.tile_pool(name="psig", bufs=3, space="PSUM") as psig,
            tc.tile_pool(name="pso", bufs=2, space="PSUM") as pso,
        ):
            xskew = pp.tile([128, T * 64], fp16, tag="xskew")
            wis_s = pp.tile([128, 512], fp16, tag="wis")
            wss0_s = pp.tile([128, 512], fp16, tag="wss0")
            wss1_s = pp.tile([128, 512], fp16, tag="wss1")
            bias_s = pp.tile([128, 4], fp32, tag="bias")
            rhs = [
                pp.tile([128, 66], fp16, tag=f"rhs{i}", name=f"rhs{i}")
                for i in range(2)
            ]
            # cbuf lives in SBUF: gpsimd (t2 = sig_f * c) cannot read PSUM,
            # and the VE c-write is cheaper against SBUF anyway.
            cbuf = pp.tile([128, 64], fp32, tag="cbuf")
            hist = pp.tile([128, H * W], fp32, tag="hist")
            warm = pp.tile([128, 1], fp32, tag="warm")
            warm2 = pp.tile([128, max(ACT_DUMW, 1)], fp32, tag="warm2")

            # --- prologue ---
            # weights first (they gate the first LDWEIGHTS), then xs chunks.
            # xs chunks and out-DMAs are issued from the (otherwise idle) sync
            # queue: gpsimd now runs t2 on the scan chain and must stay clear.
            nc.scalar.dma_start(out=wis_s, in_=wis_d.ap())
            nc.sync.dma_start(out=wss0_s, in_=wss0_d.ap())
            nc.sync.dma_start(out=wss1_s, in_=wss1_d.ap())
            nc.scalar.dma_start(out=bias_s, in_=bias_d.ap())
            for k in range(0, T, 16):
                hi = min(T, k + 16) * 64
                nc.sync.dma_start(out=xskew[:, k * 64 : hi], in_=xs_d.ap()[:, k * 64 : hi])

            # Pull the sigmoid/tanh ACT table load to the start (overlaps DMA).
            nc.vector.memset(warm, 0.0)
            nc.scalar.activation(warm, warm, AFT.Sigmoid)
            nc.scalar.activation(warm, warm, AFT.Tanh)
            nc.vector.memset(warm2, 0.0)

            nc.vector.memset(rhs[0], 0.0)
            nc.vector.memset(rhs[1], 0.0)
            nc.vector.memset(cbuf, 0.0)

            def win(t):
                # active row window: below-diagonal rows are exactly 0 (zero
                # bias) and rows with t-r > 63 are dead. r0 rounded down to
                # even keeps fp16 writes 4B-aligned. Bias path: full width.
                if use_bias:
                    return 0, 63
                r0 = 0 if t < 64 else t - 63
                r1 = t if t < 63 else 63
                return r0 & ~1, r1

            pf = [None] * T
            pig = [None] * T
            po = [None] * T

            def emit_z(t, dep=None):
                pf[t] = psf.tile([128, 64], fp32, tag="pf", name=f"pf{t}")
                pig[t] = psig.tile([128, 128], fp32, tag="pig", name=f"pig{t}")
                po[t] = pso.tile([128, 64], fp32, tag="po", name=f"po{t}")
                a, b = win(t)
                r = xskew[:, t * 64 + a : t * 64 + b + 1]
                mf = nc.tensor.matmul(pf[t][:, a : b + 1], lhsT=wis_s[:, 0:128], rhs=r,
                                      start=True, stop=False, skip_group_check=True)
                if dep is not None:
                    tile.add_dep_helper(mf.ins, dep.ins, sync=True,
                                        reason="delay z to fill PE idle tail")
                mi = nc.tensor.matmul(pig[t][:, a : b + 1], lhsT=wis_s[:, 128:256], rhs=r,
                                      start=True, stop=False, skip_group_check=True)
                mg = nc.tensor.matmul(pig[t][:, 64 + a : 64 + b + 1], lhsT=wis_s[:, 256:384], rhs=r,
                                      start=False, stop=False, skip_group_check=True)
                tile.add_dep_helper(mg.ins, mi.ins, sync=False,
                                    reason="bank-clear MM must run first")
                nc.tensor.matmul(po[t][:, a : b + 1], lhsT=wis_s[:, 384:512], rhs=r,
                                 start=True, stop=False, skip_group_check=True)

            for t in range(LOOKAHEAD):
                emit_z(t)

            prev_tanh = None
            # --- the 127-step scan (gate order: f, i, g, o) ---
            for t in range(T):
                if t + LOOKAHEAD < T:
                    emit_z(t + LOOKAHEAD, dep=prev_tanh if Z_DELAY else None)

                a, b = win(t)
                rbuf = rhs[t % 2]
                tap0 = rbuf[:, 1 + a : 2 + b]
                tap1 = rbuf[:, 2 + a : 3 + b]

                def rec(dst, q, stop):
                    nc.tensor.matmul(dst, lhsT=wss0_s[:, q * 128 : (q + 1) * 128], rhs=tap0,
                                     start=False, stop=False, skip_group_check=True)
                    return nc.tensor.matmul(dst, lhsT=wss1_s[:, q * 128 : (q + 1) * 128], rhs=tap1,
                                            start=False, stop=stop, skip_group_check=True)

                rec(pf[t][:, a : b + 1], 0, True)             # f first
                rec(pig[t][:, a : b + 1], 1, False)           # i
                rec(pig[t][:, 64 + a : 64 + b + 1], 2, True)  # g
                rec(po[t][:, a : b + 1], 3, True)             # o last

                # ACT keep-warm dummy: queued right after TANH(t-1), so it
                # runs back-to-back and covers the MM segment; sig_f then
                # issues without the pipeline-restart bubble.
                if ACT_DUMW and prev_tanh is not None and not use_bias:
                    nc.scalar.activation(warm2[:, :ACT_DUMW], warm2[:, :ACT_DUMW],
                                         AFT.Sigmoid)

                sig = gp.tile([128, 192], sig_dt, tag="sig")
                so = gp.tile([128, 64], fp16, tag="so")
                if use_bias:
                    nc.scalar.activation(sig[:, 0:64], pf[t], AFT.Sigmoid, bias=bias_s[:, 0:1])
                    nc.scalar.activation(sig[:, 64:128], pig[t][:, 0:64], AFT.Sigmoid, bias=bias_s[:, 1:2])
                    nc.scalar.activation(sig[:, 128:192], pig[t][:, 64:128], AFT.Sigmoid, bias=bias_s[:, 2:3])
                    soi = nc.scalar.activation(so, po[t], AFT.Sigmoid, bias=bias_s[:, 3:4])
                else:
                    nc.scalar.activation(sig[:, a : b + 1], pf[t][:, a : b + 1], AFT.Sigmoid)
                    nc.scalar.activation(
                        sig[:, 64:192].rearrange("p (g r) -> p g r", g=2)[:, :, a : b + 1],
                        pig[t].rearrange("p (g r) -> p g r", g=2)[:, :, a : b + 1],
                        AFT.Sigmoid,
                    )

                t1 = gp.tile([128, 64], sig_dt, tag="t1")
                t2 = gp.tile([128, 64], fp32, tag="t2")
                # t2 = sig_f * c ; t1 = (sig_g - 0.5) * sig_i = i*g/2
                t2_eng = nc.gpsimd if GP_T2 else nc.vector
                t2_eng.tensor_mul(t2[:, a : b + 1], sig[:, a : b + 1], cbuf[:, a : b + 1])
                t1i = nc.vector.scalar_tensor_tensor(
                    t1[:, a : b + 1], sig[:, 128 + a : 128 + b + 1], -0.5,
                    sig[:, 64 + a : 64 + b + 1], ALU.add, ALU.mult
                )
                # c = t1*2 + t2
                nc.vector.scalar_tensor_tensor(
                    cbuf[:, a : b + 1], t1[:, a : b + 1], 2.0, t2[:, a : b + 1],
                    ALU.mult, ALU.add
                )

                if not use_bias:
                    soi = nc.scalar.activation(so[:, a : b + 1], po[t][:, a : b + 1],
                                               AFT.Sigmoid)
                    if SIGO_DELAY:
                        tile.add_dep_helper(soi.ins, t1i.ins, sync=True,
                                            reason="delay sig_o so TANH runs b2b")

                tc_s = gp.tile([128, 64], fp16, tag="tc")
                tci = nc.scalar.activation(tc_s[:, a : b + 1], cbuf[:, a : b + 1], AFT.Tanh)
                prev_tanh = tci

                # h (fp16) into the next rhs buffer -- this is the serial chain
                nbuf = rhs[(t + 1) % 2]
                nc.vector.tensor_mul(nbuf[:, 2 + a : 3 + b], so[:, a : b + 1], tc_s[:, a : b + 1])

                # h (fp32) into unskewed history, in-band rows only (off chain)
                r0 = 0 if t < W else t - (W - 1)
                r1 = t if t < W else W - 1
                cnt = r1 - r0 + 1
                base = r0 * 63 + t
                hview = (
                    hist[:, base : base + (cnt - 1) * 63 + 1 : 63]
                    if cnt > 1
                    else hist[:, base : base + 1]
                )
                hist_eng = nc.gpsimd if GP_HIST else nc.vector
                hist_eng.tensor_mul(hview, so[:, r0 : r0 + cnt], tc_s[:, r0 : r0 + cnt])

                # epilogue overlap: rows [k0, k1) are final after step k1-1+63
                for k0, k1 in ((0, 16), (16, 32), (32, 48), (48, 56), (56, 60), (60, 64)):
                    if t == k1 - 1 + 63:
                        nc.sync.dma_start(
                            out=out_d.ap()[:, k0 * 64 : k1 * 64],
                            in_=hist[:, k0 * 64 : k1 * 64],
                        )

    nc.compile()
    return nc


def _get_program(use_bias: bool):
    if use_bias not in _PROGRAM_CACHE:
        _PROGRAM_CACHE[use_bias] = _build_program(use_bias)
    return _PROGRAM_CACHE[use_bias]


def _prep_weights(w):
    """(512, 128) -> (128, 512) fp16 with gate column order [f, i, 2g, o]."""
    wt = w.T.astype(np.float32)  # (128, 512) in [i, f, o, g] order
    out = np.concatenate(
        [wt[:, 128:256], wt[:, 0:128], 2.0 * wt[:, 384:512], wt[:, 256:384]], axis=1
    )
    return np.ascontiguousarray(out.astype(np.float16))


def kernel(x, w_is, b_is, w_ss, b_ss, _trace=False, _trace_kwargs=None):
    from concourse.bass_utils import run_bass_kernel_spmd

    x = np.asarray(x, dtype=np.float32)
    w_is = np.asarray(w_is, dtype=np.float32)
    b_is = np.asarray(b_is, dtype=np.float32)
    w_ss = np.asarray(w_ss, dtype=np.float32)
    b_ss = np.asarray(b_ss, dtype=np.float32)
    B = x.shape[0]
    assert x.shape == (B, CIN, H, W), x.shape

    bias = (b_is + b_ss).astype(np.float32)  # (512,) in [i, f, o, g] order
    use_bias = bool(np.any(bias != 0.0))
    nc = _get_program(use_bias)

    wis_h = _prep_weights(w_is)
    wss0_h = _prep_weights(w_ss[:, :, 0, 0])
    wss1_h = _prep_weights(w_ss[:, :, 1, 0])
    bq = bias.reshape(4, HID)  # [i, f, o, g]
    bias_h = np.ascontiguousarray(
        np.stack([bq[1], bq[0], 2.0 * bq[3], bq[2]], axis=1).astype(np.float32)
    )  # (128, 4) in [f, i, 2g, o] order

    # host-side skew + fp16 cast, t-major: xs[b, c, t*64 + r] = x[b, c, r, t-r]
    xs_all = np.zeros((B, CIN, T, 64), np.float16)
    x16 = x.astype(np.float16)
    for r in range(H):
        xs_all[:, :, r : r + W, r] = x16[:, :, r, :].transpose(0, 1, 2)
    xs_all = xs_all.reshape(B, CIN, T * 64)

    in_maps = []
    for b in range(N_CORES):
        in_maps.append(
            {
                "xs": np.ascontiguousarray(xs_all[b % B]),
                "wis": wis_h,
                "wss0": wss0_h,
                "wss1": wss1_h,
                "bias": bias_h,
            }
        )

    res = run_bass_kernel_spmd(
        nc,
        in_maps,
        core_ids=list(range(N_CORES)),
        trace=_trace,
        **(_trace_kwargs or {}),
    )
    out = np.stack(
        [res.results[b]["out"].reshape(HID, H, W) for b in range(B)], axis=0
    ).astype(np.float32)
    if _trace:
        return out, res
    return out


# revision 13
# speedup vs baseline: 1.0895x; 1.0895x over previous
"""DiagonalLSTM Trainium2 kernel.

Reference computation (per batch element b):
  xs = skew(x)                               # (Cin, H, 2W-1), row r shifted right by r
  z_is = w_is @ xs + b_is                    # 1x1 conv -> 4*HID channels
  for t in 0..2W-2:                          # sequential scan over skewed width
      hs[o, r] = wss[o,c,0] h[c,r-1] + wss[o,c,1] h[c,r] + b_ss[o]
      z = z_is[:, :, t] + hs
      i, f, o_, g = sig, sig, sig, tanh of the 4 gate quarters
      c = f*c + i*g ; h = o_*tanh(c)
  out = unskew(h history)

Sharding: data-parallel over batch B=8 across the 8 NeuronCores (the t-scan is
inherently sequential; each core runs its own batch element's full scan).

Per-core layout (128 partitions = channels), gate order [f, i, g, o]:
 - gates-on-partitions: per step the gates live in THREE psum banks -- f
   (128x64), i|g (128x128), o (128x64).
 - g is computed VIA SIGMOID: tanh(z) = 2*sigmoid(2z) - 1, the factor 2 folded
   into the g columns of all weights host-side.
 - Measured critical chain per step (HW trace, 2072ns at baseline):
     MM f-taps -> sig_f -> sig_ig -> t1 -> c -> TANH -> h-write
   Optimizations in this version, all aimed at that chain:
   * z_is(t+1) matmuls are DELAYED (dep on TANH(t-1)) so the PE streams them
     right up to the moment h(t-1) lands -> the first rec tap issues
     back-to-back instead of paying the isolated-matmul pipe-fill (~210ns).
   * an ACT dummy op after each TANH keeps the scalar engine's pipeline busy
     through the MM segment so sig_f starts back-to-back (saves the ~130-cycle
     restart bubble: 313ns -> ~200ns).
   * sig_o carries a dep on t1 so it issues late and TANH then runs
     back-to-back behind it on the ACT queue (314ns -> ~200ns).
   * sigmoid outputs are fp16 -> t1 runs in the DVE 2x mode.
   * t1 runs on the (otherwise idle) GPSIMD engine, shortening the VE serial
     segment to t2 -> c.
 - x is pre-skewed and pre-cast to fp16 ON THE HOST, t-major (zero padded);
   the step-t rhs is the contiguous slice xs[:, t*64:(t+1)*64].
 - h is written as fp16 into a (128 x 66) rhs buffer with cols 0:2 always 0;
   tap0 (h[r-1]) = cols 1:65 and tap1 (h[r]) = cols 2:66.
 - h history is stored fp32 directly in unskewed layout hist[c, r*64+w] via a
   stride-63 write of the in-band rows; the output DMA is chunked by row
   groups overlapping the scan tail.
 - zero-bias fast path: every per-step op covers only the ACTIVE row window
   [max(0,t-63) & ~1, min(t,63)].
"""

import sys

if "/opt/trn_rl_repo" not in sys.path:
    sys.path.insert(0, "/opt/trn_rl_repo")

import numpy as np

N_CORES = 8
HID = 128
CIN = 128
H = 64
W = 64
T = 2 * W - 1  # 127
LOOKAHEAD = 1
RCHUNK = 16  # epilogue row-chunk size

# --- scan-chain tunables ---
ACT_DUMW = 0    # width of the ACT keep-warm dummy (0 disables; the tile
                # scheduler reorders unpinned dummies and they choke ACT)
Z_DELAY = True  # delay z_is(t+1) MMs behind TANH(t) to fill the PE tail
GP_T2 = False   # t2 on vector (gpsimd cannot read PSUM, and cbuf wants PSUM)
GP_HIST = True  # write the h history on gpsimd instead of vector
SIG_FP16 = False  # stt has no 2x fp16 mode; fp32 sig is no slower
SIGO_DELAY = True  # delay sig_o behind t1 so TANH runs back-to-back

_PROGRAM_CACHE = {}


def _build_program(use_bias: bool):
    import concourse.bacc as bacc
    import concourse.tile as tile
    from concourse import mybir

    fp32 = mybir.dt.float32
    fp16 = mybir.dt.float16
    AFT = mybir.ActivationFunctionType
    ALU = mybir.AluOpType

    nc = bacc.Bacc("TRN2", debug=False, num_devices=N_CORES)
    xs_d = nc.dram_tensor("xs", [CIN, T * 64], fp16, kind="ExternalInput")
    wis_d = nc.dram_tensor("wis", [CIN, 4 * HID], fp16, kind="ExternalInput")
    wss0_d = nc.dram_tensor("wss0", [HID, 4 * HID], fp16, kind="ExternalInput")
    wss1_d = nc.dram_tensor("wss1", [HID, 4 * HID], fp16, kind="ExternalInput")
    bias_d = nc.dram_tensor("bias", [HID, 4], fp32, kind="ExternalInput")
    out_d = nc.dram_tensor("out", [HID, H * W], fp32, kind="ExternalOutput")

    sig_dt = fp16 if SIG_FP16 else fp32

    with tile.TileContext(nc) as tc:
        with (
            tc.tile_pool(name="persist", bufs=1) as pp,
            tc.tile_pool(name="gates", bufs=3) as gp,
            tc.tile_pool(name="psf", bufs=2, space="PSUM") as psf,
            tc.tile_pool(name="psc", bufs=1, space="PSUM") as psc,
            tc.tile_pool(name="psig", bufs=3, space="PSUM") as psig,
            tc.tile_pool(name="pso", bufs=2, space="PSUM") as pso,
        ):
            xskew = pp.tile([128, T * 64], fp16, tag="xskew")
            wis_s = pp.tile([128, 512], fp16, tag="wis")
            wss0_s = pp.tile([128, 512], fp16, tag="wss0")
            wss1_s = pp.tile([128, 512], fp16, tag="wss1")
            bias_s = pp.tile([128, 4], fp32, tag="bias")
            rhs = [
                pp.tile([128, 66], fp16, tag=f"rhs{i}", name=f"rhs{i}")
                for i in range(2)
            ]
            # cbuf in PSUM: TANH then reads the same space sig_o does, which
            # keeps the delayed-sig_o -> TANH pair back-to-back at the cheap
            # (172+FD)-cycle cost.
            cbuf = psc.tile([128, 64], fp32, tag="cbuf")
            hist = pp.tile([128, H * W], fp32, tag="hist")
            warm = pp.tile([128, 1], fp32, tag="warm")
            warm2 = pp.tile([128, max(ACT_DUMW, 1)], fp32, tag="warm2")

            # --- prologue ---
            # weights first (they gate the first LDWEIGHTS), then xs chunks.
            # xs chunks and out-DMAs are issued from the (otherwise idle) sync
            # queue: gpsimd now runs t2 on the scan chain and must stay clear.
            nc.scalar.dma_start(out=wis_s, in_=wis_d.ap())
            nc.sync.dma_start(out=wss0_s, in_=wss0_d.ap())
            nc.sync.dma_start(out=wss1_s, in_=wss1_d.ap())
            nc.scalar.dma_start(out=bias_s, in_=bias_d.ap())
            for k in range(0, T, 16):
                hi = min(T, k + 16) * 64
                nc.sync.dma_start(out=xskew[:, k * 64 : hi], in_=xs_d.ap()[:, k * 64 : hi])

            # Pull the sigmoid/tanh ACT table load to the start (overlaps DMA).
            nc.vector.memset(warm, 0.0)
            nc.scalar.activation(warm, warm, AFT.Sigmoid)
            nc.scalar.activation(warm, warm, AFT.Tanh)
            nc.vector.memset(warm2, 0.0)

            nc.vector.memset(rhs[0], 0.0)
            nc.vector.memset(rhs[1], 0.0)
            nc.vector.memset(cbuf, 0.0)

            def win(t):
                # active row window: below-diagonal rows are exactly 0 (zero
                # bias) and rows with t-r > 63 are dead. r0 rounded down to
                # even keeps fp16 writes 4B-aligned. Bias path: full width.
                if use_bias:
                    return 0, 63
                r0 = 0 if t < 64 else t - 63
                r1 = t if t < 63 else 63
                return r0 & ~1, r1

            pf = [None] * T
            pig = [None] * T
            po = [None] * T

            def emit_z(t, dep=None):
                pf[t] = psf.tile([128, 64], fp32, tag="pf", name=f"pf{t}")
                pig[t] = psig.tile([128, 128], fp32, tag="pig", name=f"pig{t}")
                po[t] = pso.tile([128, 64], fp32, tag="po", name=f"po{t}")
                a, b = win(t)
                r = xskew[:, t * 64 + a : t * 64 + b + 1]
                mf = nc.tensor.matmul(pf[t][:, a : b + 1], lhsT=wis_s[:, 0:128], rhs=r,
                                      start=True, stop=False, skip_group_check=True)
                if dep is not None:
                    tile.add_dep_helper(mf.ins, dep.ins, sync=True,
                                        reason="delay z to fill PE idle tail")
                mi = nc.tensor.matmul(pig[t][:, a : b + 1], lhsT=wis_s[:, 128:256], rhs=r,
                                      start=True, stop=False, skip_group_check=True)
                mg = nc.tensor.matmul(pig[t][:, 64 + a : 64 + b + 1], lhsT=wis_s[:, 256:384], rhs=r,
                                      start=False, stop=False, skip_group_check=True)
                tile.add_dep_helper(mg.ins, mi.ins, sync=False,
                                    reason="bank-clear MM must run first")
                mo = nc.tensor.matmul(po[t][:, a : b + 1], lhsT=wis_s[:, 384:512], rhs=r,
                                      start=True, stop=False, skip_group_check=True)
                return mo

            last_z = emit_z(0)

            prev_tanh = None
            # --- the 127-step scan (gate order: f, i, g, o) ---
            for t in range(T):
                a, b = win(t)
                rbuf = rhs[t % 2]
                tap0 = rbuf[:, 1 + a : 2 + b]
                tap1 = rbuf[:, 2 + a : 3 + b]

                def rec(dst, q, stop, pin_after=None):
                    m0 = nc.tensor.matmul(dst, lhsT=wss0_s[:, q * 128 : (q + 1) * 128], rhs=tap0,
                                          start=False, stop=False, skip_group_check=True)
                    if pin_after is not None:
                        # queue-order pin: the first rec tap issues right
                        # behind the (tanh-delayed) z matmuls, so it streams
                        # back-to-back instead of paying the isolated
                        # pipe-fill after a PE idle.
                        tile.add_dep_helper(m0.ins, pin_after.ins, sync=False,
                                            reason="rec taps queue behind z filler")
                    return nc.tensor.matmul(dst, lhsT=wss1_s[:, q * 128 : (q + 1) * 128], rhs=tap1,
                                            start=False, stop=stop, skip_group_check=True)

                rec(pf[t][:, a : b + 1], 0, True,
                    pin_after=last_z if Z_DELAY else None)    # f first
                rec(pig[t][:, a : b + 1], 1, False)           # i
                rec(pig[t][:, 64 + a : 64 + b + 1], 2, True)  # g
                rec(po[t][:, a : b + 1], 3, True)             # o last

                # ACT keep-warm dummy: queued right after TANH(t-1), so it
                # runs back-to-back and covers the MM segment; sig_f then
                # issues without the pipeline-restart bubble.
                if ACT_DUMW and prev_tanh is not None and not use_bias:
                    nc.scalar.activation(warm2[:, :ACT_DUMW], warm2[:, :ACT_DUMW],
                                         AFT.Sigmoid)

                sig = gp.tile([128, 192], sig_dt, tag="sig")
                so = gp.tile([128, 64], fp16, tag="so")
                if use_bias:
                    nc.scalar.activation(sig[:, 0:64], pf[t], AFT.Sigmoid, bias=bias_s[:, 0:1])
                    nc.scalar.activation(sig[:, 64:128], pig[t][:, 0:64], AFT.Sigmoid, bias=bias_s[:, 1:2])
                    nc.scalar.activation(sig[:, 128:192], pig[t][:, 64:128], AFT.Sigmoid, bias=bias_s[:, 2:3])
                    soi = nc.scalar.activation(so, po[t], AFT.Sigmoid, bias=bias_s[:, 3:4])
                else:
                    nc.scalar.activation(sig[:, a : b + 1], pf[t][:, a : b + 1], AFT.Sigmoid)
                    nc.scalar.activation(
                        sig[:, 64:192].rearrange("p (g r) -> p g r", g=2)[:, :, a : b + 1],
                        pig[t].rearrange("p (g r) -> p g r", g=2)[:, :, a : b + 1],
                        AFT.Sigmoid,
                    )

                t1 = gp.tile([128, 64], sig_dt, tag="t1")
                t2 = gp.tile([128, 64], fp32, tag="t2")
                # t2 = sig_f * c ; t1 = (sig_g - 0.5) * sig_i = i*g/2
                t2_eng = nc.gpsimd if GP_T2 else nc.vector
                t2_eng.tensor_mul(t2[:, a : b + 1], sig[:, a : b + 1], cbuf[:, a : b + 1])
                t1i = nc.vector.scalar_tensor_tensor(
                    t1[:, a : b + 1], sig[:, 128 + a : 128 + b + 1], -0.5,
                    sig[:, 64 + a : 64 + b + 1], ALU.add, ALU.mult
                )
                # c = t1*2 + t2
                nc.vector.scalar_tensor_tensor(
                    cbuf[:, a : b + 1], t1[:, a : b + 1], 2.0, t2[:, a : b + 1],
                    ALU.mult, ALU.add
                )

                if not use_bias:
                    soi = nc.scalar.activation(so[:, a : b + 1], po[t][:, a : b + 1],
                                               AFT.Sigmoid)
                    if SIGO_DELAY:
                        tile.add_dep_helper(soi.ins, t1i.ins, sync=True,
                                            reason="delay sig_o so TANH runs b2b")

                tc_s = gp.tile([128, 64], fp16, tag="tc")
                tci = nc.scalar.activation(tc_s[:, a : b + 1], cbuf[:, a : b + 1], AFT.Tanh)
                prev_tanh = tci

                # emit next step's z_is now, gated on this step's TANH: the
                # PE then streams them right up to the h(t) hand-off and the
                # next rec taps issue back-to-back behind them.
                if t + 1 < T:
                    last_z = emit_z(t + 1, dep=prev_tanh if Z_DELAY else None)

                # h (fp16) into the next rhs buffer -- this is the serial chain
                nbuf = rhs[(t + 1) % 2]
                nc.vector.tensor_mul(nbuf[:, 2 + a : 3 + b], so[:, a : b + 1], tc_s[:, a : b + 1])

                # h (fp32) into unskewed history, in-band rows only (off chain)
                r0 = 0 if t < W else t - (W - 1)
                r1 = t if t < W else W - 1
                cnt = r1 - r0 + 1
                base = r0 * 63 + t
                hview = (
                    hist[:, base : base + (cnt - 1) * 63 + 1 : 63]
                    if cnt > 1
                    else hist[:, base : base + 1]
                )
                hist_eng = nc.gpsimd if GP_HIST else nc.vector
                hist_eng.tensor_mul(hview, so[:, r0 : r0 + cnt], tc_s[:, r0 : r0 + cnt])

                # epilogue overlap: rows [k0, k1) are final after step k1-1+63
                for k0, k1 in ((0, 16), (16, 32), (32, 48), (48, 56), (56, 60), (60, 64)):
                    if t == k1 - 1 + 63:
                        nc.sync.dma_start(
                            out=out_d.ap()[:, k0 * 64 : k1 * 64],
                            in_=hist[:, k0 * 64 : k1 * 64],
                        )

    nc.compile()
    return nc


def _get_program(use_bias: bool):
    if use_bias not in _PROGRAM_CACHE:
        _PROGRAM_CACHE[use_bias] = _build_program(use_bias)
    return _PROGRAM_CACHE[use_bias]


def _prep_weights(w):
    """(512, 128) -> (128, 512) fp16 with gate column order [f, i, 2g, o]."""
    wt = w.T.astype(np.float32)  # (128, 512) in [i, f, o, g] order
    out = np.concatenate(
        [wt[:, 128:256], wt[:, 0:128], 2.0 * wt[:, 384:512], wt[:, 256:384]], axis=1
    )
    return np.ascontiguousarray(out.astype(np.float16))


def kernel(x, w_is, b_is, w_ss, b_ss, _trace=False, _trace_kwargs=None):
    from concourse.bass_utils import run_bass_kernel_spmd

    x = np.asarray(x, dtype=np.float32)
    w_is = np.asarray(w_is, dtype=np.float32)
    b_is = np.asarray(b_is, dtype=np.float32)
    w_ss = np.asarray(w_ss, dtype=np.float32)
    b_ss = np.asarray(b_ss, dtype=np.float32)
    B = x.shape[0]
    assert x.shape == (B, CIN, H, W), x.shape

    bias = (b_is + b_ss).astype(np.float32)  # (512,) in [i, f, o, g] order
    use_bias = bool(np.any(bias != 0.0))
    nc = _get_program(use_bias)

    wis_h = _prep_weights(w_is)
    wss0_h = _prep_weights(w_ss[:, :, 0, 0])
    wss1_h = _prep_weights(w_ss[:, :, 1, 0])
    bq = bias.reshape(4, HID)  # [i, f, o, g]
    bias_h = np.ascontiguousarray(
        np.stack([bq[1], bq[0], 2.0 * bq[3], bq[2]], axis=1).astype(np.float32)
    )  # (128, 4) in [f, i, 2g, o] order

    # host-side skew + fp16 cast, t-major: xs[b, c, t*64 + r] = x[b, c, r, t-r]
    xs_all = np.zeros((B, CIN, T, 64), np.float16)
    x16 = x.astype(np.float16)
    for r in range(H):
        xs_all[:, :, r : r + W, r] = x16[:, :, r, :].transpose(0, 1, 2)
    xs_all = xs_all.reshape(B, CIN, T * 64)

    in_maps = []
    for b in range(N_CORES):
        in_maps.append(
            {
                "xs": np.ascontiguousarray(xs_all[b % B]),
                "wis": wis_h,
                "wss0": wss0_h,
                "wss1": wss1_h,
                "bias": bias_h,
            }
        )

    res = run_bass_kernel_spmd(
        nc,
        in_maps,
        core_ids=list(range(N_CORES)),
        trace=_trace,
        **(_trace_kwargs or {}),
    )
    out = np.stack(
        [res.results[b]["out"].reshape(HID, H, W) for b in range(B)], axis=0
    ).astype(np.float32)
    if _trace:
        return out, res
    return out


# revision 18
# speedup vs baseline: 1.0956x; 1.0057x over previous
"""DiagonalLSTM Trainium2 kernel.

Reference computation (per batch element b):
  xs = skew(x)                               # (Cin, H, 2W-1), row r shifted right by r
  z_is = w_is @ xs + b_is                    # 1x1 conv -> 4*HID channels
  for t in 0..2W-2:                          # sequential scan over skewed width
      hs[o, r] = wss[o,c,0] h[c,r-1] + wss[o,c,1] h[c,r] + b_ss[o]
      z = z_is[:, :, t] + hs
      i, f, o_, g = sig, sig, sig, tanh of the 4 gate quarters
      c = f*c + i*g ; h = o_*tanh(c)
  out = unskew(h history)

Sharding: data-parallel over batch B=8 across the 8 NeuronCores (the t-scan is
inherently sequential; each core runs its own batch element's full scan).

Per-core layout (128 partitions = channels), gate order [f, i, g, o]:
 - gates-on-partitions: per step the gates live in THREE psum banks -- f
   (128x64), i|g (128x128), o (128x64).
 - g is computed VIA SIGMOID: tanh(z) = 2*sigmoid(2z) - 1, the factor 2 folded
   into the g columns of all weights host-side.
 - Measured critical chain per step (HW trace, 2072ns at baseline):
     MM f-taps -> sig_f -> sig_ig -> t1 -> c -> TANH -> h-write
   Optimizations in this version, all aimed at that chain:
   * z_is(t+1) matmuls are DELAYED (dep on TANH(t-1)) so the PE streams them
     right up to the moment h(t-1) lands -> the first rec tap issues
     back-to-back instead of paying the isolated-matmul pipe-fill (~210ns).
   * an ACT dummy op after each TANH keeps the scalar engine's pipeline busy
     through the MM segment so sig_f starts back-to-back (saves the ~130-cycle
     restart bubble: 313ns -> ~200ns).
   * sig_o carries a dep on t1 so it issues late and TANH then runs
     back-to-back behind it on the ACT queue (314ns -> ~200ns).
   * sigmoid outputs are fp16 -> t1 runs in the DVE 2x mode.
   * t1 runs on the (otherwise idle) GPSIMD engine, shortening the VE serial
     segment to t2 -> c.
 - x is pre-skewed and pre-cast to fp16 ON THE HOST, t-major (zero padded);
   the step-t rhs is the contiguous slice xs[:, t*64:(t+1)*64].
 - h is written as fp16 into a (128 x 66) rhs buffer with cols 0:2 always 0;
   tap0 (h[r-1]) = cols 1:65 and tap1 (h[r]) = cols 2:66.
 - h history is stored fp32 directly in unskewed layout hist[c, r*64+w] via a
   stride-63 write of the in-band rows; the output DMA is chunked by row
   groups overlapping the scan tail.
 - zero-bias fast path: every per-step op covers only the ACTIVE row window
   [max(0,t-63) & ~1, min(t,63)].
"""

import sys

if "/opt/trn_rl_repo" not in sys.path:
    sys.path.insert(0, "/opt/trn_rl_repo")

import numpy as np

N_CORES = 8
HID = 128
CIN = 128
H = 64
W = 64
T = 2 * W - 1  # 127
LOOKAHEAD = 1
RCHUNK = 16  # epilogue row-chunk size

# --- scan-chain tunables ---
# Measured law (HW trace): a chain instruction waiting on fresh data always
# pays its full isolated-entry cost (pipe fill + access restart); only an op
# that was ISSUABLE before its engine went idle gets back-to-back pricing.
# So each isolated chain entry gets a PRE-FILLER: a scratch op on the same
# engine, dep-pinned to start one chain-unit early and sized to still be
# running when the real op's data lands.
PRE_MM_W = 96   # pre-matmul filler width (0 disables)
PRE_TANH_W = 1  # pre-tanh filler width (0 disables)
PRE_SF_W = 0    # pre-sigmoid_f filler width (0 disables; costs ACT busy)
GP_HIST = True  # write the h history on gpsimd instead of vector
SIG_FP16 = False  # stt has no 2x fp16 mode; fp32 sig is no slower

_PROGRAM_CACHE = {}


def _build_program(use_bias: bool):
    import concourse.bacc as bacc
    import concourse.tile as tile
    from concourse import mybir

    fp32 = mybir.dt.float32
    fp16 = mybir.dt.float16
    AFT = mybir.ActivationFunctionType
    ALU = mybir.AluOpType

    nc = bacc.Bacc("TRN2", debug=False, num_devices=N_CORES)
    xs_d = nc.dram_tensor("xs", [CIN, T * 64], fp16, kind="ExternalInput")
    wis_d = nc.dram_tensor("wis", [CIN, 4 * HID], fp16, kind="ExternalInput")
    wss0_d = nc.dram_tensor("wss0", [HID, 4 * HID], fp16, kind="ExternalInput")
    wss1_d = nc.dram_tensor("wss1", [HID, 4 * HID], fp16, kind="ExternalInput")
    bias_d = nc.dram_tensor("bias", [HID, 4], fp32, kind="ExternalInput")
    out_d = nc.dram_tensor("out", [HID, H * W], fp32, kind="ExternalOutput")

    sig_dt = fp16 if SIG_FP16 else fp32

    with tile.TileContext(nc) as tc:
        with (
            tc.tile_pool(name="persist", bufs=1) as pp,
            tc.tile_pool(name="gates", bufs=3) as gp,
            tc.tile_pool(name="psf", bufs=2, space="PSUM") as psf,
            tc.tile_pool(name="psd", bufs=1, space="PSUM") as psd,
            tc.tile_pool(name="psig", bufs=3, space="PSUM") as psig,
            tc.tile_pool(name="pso", bufs=2, space="PSUM") as pso,
        ):
            xskew = pp.tile([128, T * 64], fp16, tag="xskew")
            wis_s = pp.tile([128, 512], fp16, tag="wis")
            wss0_s = pp.tile([128, 512], fp16, tag="wss0")
            wss1_s = pp.tile([128, 512], fp16, tag="wss1")
            bias_s = pp.tile([128, 4], fp32, tag="bias")
            rhs = [
                pp.tile([128, 66], fp16, tag=f"rhs{i}", name=f"rhs{i}")
                for i in range(2)
            ]
            # cbuf in SBUF; the 8th PSUM bank goes to the pre-filler scratch.
            cbuf = pp.tile([128, 64], fp32, tag="cbuf")
            dumm_ps = psd.tile([128, max(PRE_MM_W, 96)], fp32, tag="dummps")
            scr_s = pp.tile([128, 8], fp32, tag="scrs")
            hist = pp.tile([128, H * W], fp32, tag="hist")
            warm = pp.tile([128, 1], fp32, tag="warm")
            warm2 = pp.tile([128, 64], fp32, tag="warm2")

            # --- prologue ---
            # weights first (they gate the first LDWEIGHTS), then xs chunks.
            # xs chunks and out-DMAs are issued from the (otherwise idle) sync
            # queue: gpsimd now runs t2 on the scan chain and must stay clear.
            nc.scalar.dma_start(out=wis_s, in_=wis_d.ap())
            nc.sync.dma_start(out=wss0_s, in_=wss0_d.ap())
            nc.sync.dma_start(out=wss1_s, in_=wss1_d.ap())
            nc.scalar.dma_start(out=bias_s, in_=bias_d.ap())
            for k in range(0, T, 16):
                hi = min(T, k + 16) * 64
                nc.sync.dma_start(out=xskew[:, k * 64 : hi], in_=xs_d.ap()[:, k * 64 : hi])

            # Pull the sigmoid/tanh ACT table load to the start (overlaps DMA).
            nc.vector.memset(warm, 0.0)
            nc.scalar.activation(warm, warm, AFT.Sigmoid)
            nc.scalar.activation(warm, warm, AFT.Tanh)
            nc.vector.memset(warm2, 0.0)

            nc.vector.memset(rhs[0], 0.0)
            nc.vector.memset(rhs[1], 0.0)
            nc.vector.memset(cbuf, 0.0)

            def win(t):
                # active row window: below-diagonal rows are exactly 0 (zero
                # bias) and rows with t-r > 63 are dead. r0 rounded down to
                # even keeps fp16 writes 4B-aligned. Bias path: full width.
                if use_bias:
                    return 0, 63
                r0 = 0 if t < 64 else t - 63
                r1 = t if t < 63 else 63
                return r0 & ~1, r1

            pf = [None] * T
            pig = [None] * T
            po = [None] * T

            def emit_z(t, pin_after=None):
                pf[t] = psf.tile([128, 64], fp32, tag="pf", name=f"pf{t}")
                pig[t] = psig.tile([128, 128], fp32, tag="pig", name=f"pig{t}")
                po[t] = pso.tile([128, 64], fp32, tag="po", name=f"po{t}")
                a, b = win(t)
                r = xskew[:, t * 64 + a : t * 64 + b + 1]
                mf = nc.tensor.matmul(pf[t][:, a : b + 1], lhsT=wis_s[:, 0:128], rhs=r,
                                      start=True, stop=False, skip_group_check=True)
                if pin_after is not None:
                    tile.add_dep_helper(mf.ins, pin_after.ins, sync=False,
                                        reason="z runs right behind rec taps")
                mi = nc.tensor.matmul(pig[t][:, a : b + 1], lhsT=wis_s[:, 128:256], rhs=r,
                                      start=True, stop=False, skip_group_check=True)
                tile.add_dep_helper(mi.ins, mf.ins, sync=False, reason="z b2b chain")
                mg = nc.tensor.matmul(pig[t][:, 64 + a : 64 + b + 1], lhsT=wis_s[:, 256:384], rhs=r,
                                      start=False, stop=False, skip_group_check=True)
                tile.add_dep_helper(mg.ins, mi.ins, sync=False,
                                    reason="bank-clear MM must run first")
                mo = nc.tensor.matmul(po[t][:, a : b + 1], lhsT=wis_s[:, 384:512], rhs=r,
                                      start=True, stop=False, skip_group_check=True)
                tile.add_dep_helper(mo.ins, mg.ins, sync=False, reason="z b2b chain")
                return mo

            emit_z(0)

            prev_tanh = None
            pre_mm = None
            # --- the 127-step scan (gate order: f, i, g, o) ---
            for t in range(T):
                a, b = win(t)
                rbuf = rhs[t % 2]
                tap0 = rbuf[:, 1 + a : 2 + b]
                tap1 = rbuf[:, 2 + a : 3 + b]

                def rec(dst, q, stop, pin_after=None):
                    m0 = nc.tensor.matmul(dst, lhsT=wss0_s[:, q * 128 : (q + 1) * 128], rhs=tap0,
                                          start=False, stop=False, skip_group_check=True)
                    if pin_after is not None:
                        # queue-order pin: the first rec tap issues right
                        # behind the still-streaming pre-filler matmul, so it
                        # gets back-to-back pricing instead of the isolated
                        # pipe-fill.
                        tile.add_dep_helper(m0.ins, pin_after.ins, sync=False,
                                            reason="rec taps queue behind pre-MM filler")
                    return nc.tensor.matmul(dst, lhsT=wss1_s[:, q * 128 : (q + 1) * 128], rhs=tap1,
                                            start=False, stop=stop, skip_group_check=True)

                rec(pf[t][:, a : b + 1], 0, True, pin_after=pre_mm)  # f first
                rec(pig[t][:, a : b + 1], 1, False)           # i
                rec(pig[t][:, 64 + a : 64 + b + 1], 2, True)  # g
                mo_rec = rec(po[t][:, a : b + 1], 3, True)    # o last

                # next step's z_is: ready as soon as the psum banks free up,
                # pinned right behind this step's rec taps so they stream
                # back-to-back into the PE idle window (all off-chain).
                if t + 1 < T:
                    emit_z(t + 1, pin_after=mo_rec)

                sig = gp.tile([128, 192], sig_dt, tag="sig")
                so = gp.tile([128, 64], fp16, tag="so")
                if use_bias:
                    nc.scalar.activation(sig[:, 0:64], pf[t], AFT.Sigmoid, bias=bias_s[:, 0:1])
                    nc.scalar.activation(sig[:, 64:128], pig[t][:, 0:64], AFT.Sigmoid, bias=bias_s[:, 1:2])
                    nc.scalar.activation(sig[:, 128:192], pig[t][:, 64:128], AFT.Sigmoid, bias=bias_s[:, 2:3])
                    nc.scalar.activation(so, po[t], AFT.Sigmoid, bias=bias_s[:, 3:4])
                else:
                    sfi = nc.scalar.activation(sig[:, a : b + 1], pf[t][:, a : b + 1], AFT.Sigmoid)
                    nc.scalar.activation(
                        sig[:, 64:192].rearrange("p (g r) -> p g r", g=2)[:, :, a : b + 1],
                        pig[t].rearrange("p (g r) -> p g r", g=2)[:, :, a : b + 1],
                        AFT.Sigmoid,
                    )
                    nc.scalar.activation(so[:, a : b + 1], po[t][:, a : b + 1],
                                         AFT.Sigmoid)

                t1 = gp.tile([128, 64], sig_dt, tag="t1")
                t2 = gp.tile([128, 64], fp32, tag="t2")
                # t2 = sig_f * c ; t1 = (sig_g - 0.5) * sig_i = i*g/2
                nc.vector.tensor_mul(t2[:, a : b + 1], sig[:, a : b + 1], cbuf[:, a : b + 1])
                t1i = nc.vector.scalar_tensor_tensor(
                    t1[:, a : b + 1], sig[:, 128 + a : 128 + b + 1], -0.5,
                    sig[:, 64 + a : 64 + b + 1], ALU.add, ALU.mult
                )
                # c = t1*2 + t2
                nc.vector.scalar_tensor_tensor(
                    cbuf[:, a : b + 1], t1[:, a : b + 1], 2.0, t2[:, a : b + 1],
                    ALU.mult, ALU.add
                )

                tc_s = gp.tile([128, 64], fp16, tag="tc")
                if PRE_TANH_W and not use_bias:
                    # pre-TANH filler: starts one chain-unit early (after t1)
                    # and is still draining when c lands, so the real TANH
                    # gets back-to-back pricing.
                    pti = nc.scalar.activation(scr_s[:, 0:PRE_TANH_W],
                                               warm2[:, 0:PRE_TANH_W], AFT.Tanh)
                    tile.add_dep_helper(pti.ins, t1i.ins, sync=True,
                                        reason="pre-TANH starts at t1")
                tci = nc.scalar.activation(tc_s[:, a : b + 1], cbuf[:, a : b + 1], AFT.Tanh)
                if PRE_TANH_W and not use_bias:
                    tile.add_dep_helper(tci.ins, pti.ins, sync=False,
                                        reason="TANH right behind its pre-filler")
                prev_tanh = tci

                # pre-MM filler for the NEXT step: starts when this TANH is
                # done and streams through the h hand-off.
                if PRE_MM_W and t + 1 < T and not use_bias:
                    pre_mm = nc.tensor.matmul(
                        dumm_ps[:, 0:PRE_MM_W], lhsT=wss0_s[:, 0:128],
                        rhs=xskew[:, 0:PRE_MM_W],
                        start=True, stop=True, skip_group_check=True)
                    tile.add_dep_helper(pre_mm.ins, prev_tanh.ins, sync=True,
                                        reason="pre-MM starts at TANH")
                else:
                    pre_mm = None

                # h (fp16) into the next rhs buffer -- this is the serial chain
                nbuf = rhs[(t + 1) % 2]
                nc.vector.tensor_mul(nbuf[:, 2 + a : 3 + b], so[:, a : b + 1], tc_s[:, a : b + 1])

                # h (fp32) into unskewed history, in-band rows only (off chain)
                r0 = 0 if t < W else t - (W - 1)
                r1 = t if t < W else W - 1
                cnt = r1 - r0 + 1
                base = r0 * 63 + t
                hview = (
                    hist[:, base : base + (cnt - 1) * 63 + 1 : 63]
                    if cnt > 1
                    else hist[:, base : base + 1]
                )
                hist_eng = nc.gpsimd if GP_HIST else nc.vector
                hist_eng.tensor_mul(hview, so[:, r0 : r0 + cnt], tc_s[:, r0 : r0 + cnt])

                # epilogue overlap: rows [k0, k1) are final after step k1-1+63
                for k0, k1 in ((0, 16), (16, 32), (32, 48), (48, 56), (56, 60), (60, 64)):
                    if t == k1 - 1 + 63:
                        nc.sync.dma_start(
                            out=out_d.ap()[:, k0 * 64 : k1 * 64],
                            in_=hist[:, k0 * 64 : k1 * 64],
                        )

    nc.compile()
    return nc


def _get_program(use_bias: bool):
    if use_bias not in _PROGRAM_CACHE:
        _PROGRAM_CACHE[use_bias] = _build_program(use_bias)
    return _PROGRAM_CACHE[use_bias]


def _prep_weights(w):
    """(512, 128) -> (128, 512) fp16 with gate column order [f, i, 2g, o]."""
    wt = w.T.astype(np.float32)  # (128, 512) in [i, f, o, g] order
    out = np.concatenate(
        [wt[:, 128:256], wt[:, 0:128], 2.0 * wt[:, 384:512], wt[:, 256:384]], axis=1
    )
    return np.ascontiguousarray(out.astype(np.float16))


def kernel(x, w_is, b_is, w_ss, b_ss, _trace=False, _trace_kwargs=None):
    from concourse.bass_utils import run_bass_kernel_spmd

    x = np.asarray(x, dtype=np.float32)
    w_is = np.asarray(w_is, dtype=np.float32)
    b_is = np.asarray(b_is, dtype=np.float32)
    w_ss = np.asarray(w_ss, dtype=np.float32)
    b_ss = np.asarray(b_ss, dtype=np.float32)
    B = x.shape[0]
    assert x.shape == (B, CIN, H, W), x.shape

    bias = (b_is + b_ss).astype(np.float32)  # (512,) in [i, f, o, g] order
    use_bias = bool(np.any(bias != 0.0))
    nc = _get_program(use_bias)

    wis_h = _prep_weights(w_is)
    wss0_h = _prep_weights(w_ss[:, :, 0, 0])
    wss1_h = _prep_weights(w_ss[:, :, 1, 0])
    bq = bias.reshape(4, HID)  # [i, f, o, g]
    bias_h = np.ascontiguousarray(
        np.stack([bq[1], bq[0], 2.0 * bq[3], bq[2]], axis=1).astype(np.float32)
    )  # (128, 4) in [f, i, 2g, o] order

    # host-side skew + fp16 cast, t-major: xs[b, c, t*64 + r] = x[b, c, r, t-r]
    xs_all = np.zeros((B, CIN, T, 64), np.float16)
    x16 = x.astype(np.float16)
    for r in range(H):
        xs_all[:, :, r : r + W, r] = x16[:, :, r, :].transpose(0, 1, 2)
    xs_all = xs_all.reshape(B, CIN, T * 64)

    in_maps = []
    for b in range(N_CORES):
        in_maps.append(
            {
                "xs": np.ascontiguousarray(xs_all[b % B]),
                "wis": wis_h,
                "wss0": wss0_h,
                "wss1": wss1_h,
                "bias": bias_h,
            }
        )

    res = run_bass_kernel_spmd(
        nc,
        in_maps,
        core_ids=list(range(N_CORES)),
        trace=_trace,
        **(_trace_kwargs or {}),
    )
    out = np.stack(
        [res.results[b]["out"].reshape(HID, H, W) for b in range(B)], axis=0
    ).astype(np.float32)
    if _trace:
        return out, res
    return out


# revision 24
# speedup vs baseline: 1.1022x; 1.0060x over previous
"""DiagonalLSTM Trainium2 kernel.

Reference computation (per batch element b):
  xs = skew(x)                               # (Cin, H, 2W-1), row r shifted right by r
  z_is = w_is @ xs + b_is                    # 1x1 conv -> 4*HID channels
  for t in 0..2W-2:                          # sequential scan over skewed width
      hs[o, r] = wss[o,c,0] h[c,r-1] + wss[o,c,1] h[c,r] + b_ss[o]
      z = z_is[:, :, t] + hs
      i, f, o_, g = sig, sig, sig, tanh of the 4 gate quarters
      c = f*c + i*g ; h = o_*tanh(c)
  out = unskew(h history)

Sharding: data-parallel over batch B=8 across the 8 NeuronCores (the t-scan is
inherently sequential; each core runs its own batch element's full scan).

Per-core layout (128 partitions = channels), gate order [f, i, g, o]:
 - gates-on-partitions: per step the gates live in THREE psum banks -- f
   (128x64), i|g (128x128), o (128x64).
 - g is computed VIA SIGMOID: tanh(z) = 2*sigmoid(2z) - 1, the factor 2 folded
   into the g columns of all weights host-side.
 - Measured critical chain per step (HW trace, 2072ns at baseline):
     MM f-taps -> sig_f -> sig_ig -> t1 -> c -> TANH -> h-write
   Optimizations in this version, all aimed at that chain:
   * z_is(t+1) matmuls are DELAYED (dep on TANH(t-1)) so the PE streams them
     right up to the moment h(t-1) lands -> the first rec tap issues
     back-to-back instead of paying the isolated-matmul pipe-fill (~210ns).
   * an ACT dummy op after each TANH keeps the scalar engine's pipeline busy
     through the MM segment so sig_f starts back-to-back (saves the ~130-cycle
     restart bubble: 313ns -> ~200ns).
   * sig_o carries a dep on t1 so it issues late and TANH then runs
     back-to-back behind it on the ACT queue (314ns -> ~200ns).
   * sigmoid outputs are fp16 -> t1 runs in the DVE 2x mode.
   * t1 runs on the (otherwise idle) GPSIMD engine, shortening the VE serial
     segment to t2 -> c.
 - x is pre-skewed and pre-cast to fp16 ON THE HOST, t-major (zero padded);
   the step-t rhs is the contiguous slice xs[:, t*64:(t+1)*64].
 - h is written as fp16 into a (128 x 66) rhs buffer with cols 0:2 always 0;
   tap0 (h[r-1]) = cols 1:65 and tap1 (h[r]) = cols 2:66.
 - h history is stored fp32 directly in unskewed layout hist[c, r*64+w] via a
   stride-63 write of the in-band rows; the output DMA is chunked by row
   groups overlapping the scan tail.
 - zero-bias fast path: every per-step op covers only the ACTIVE row window
   [max(0,t-63) & ~1, min(t,63)].
"""

import sys

if "/opt/trn_rl_repo" not in sys.path:
    sys.path.insert(0, "/opt/trn_rl_repo")

import numpy as np

N_CORES = 8
HID = 128
CIN = 128
H = 64
W = 64
T = 2 * W - 1  # 127
LOOKAHEAD = 1
RCHUNK = 16  # epilogue row-chunk size

# --- scan-chain tunables ---
# Measured law (HW trace): a chain instruction waiting on fresh data always
# pays its full isolated-entry cost (pipe fill + access restart); only an op
# that was ISSUABLE before its engine went idle gets back-to-back pricing.
# So each isolated chain entry gets a PRE-FILLER: a scratch op on the same
# engine, dep-pinned to start one chain-unit early and sized to still be
# running when the real op's data lands.
PRE_MM_W = 0    # pre-matmul filler width (0 disables; measured useless — the
                # PE pays its ~200ns fill after a fresh sem regardless)
PRE_TANH = True  # stretch-filler on ACT so the real TANH prices back-to-back
GP_HIST = True  # write the h history on gpsimd instead of vector
SIG_FP16 = False  # stt has no 2x fp16 mode; fp32 sig is no slower

_PROGRAM_CACHE = {}


def _build_program(use_bias: bool):
    import concourse.bacc as bacc
    import concourse.tile as tile
    from concourse import mybir

    fp32 = mybir.dt.float32
    fp16 = mybir.dt.float16
    AFT = mybir.ActivationFunctionType
    ALU = mybir.AluOpType

    nc = bacc.Bacc("TRN2", debug=False, num_devices=N_CORES)
    xs_d = nc.dram_tensor("xs", [CIN, T * 64], fp16, kind="ExternalInput")
    wis_d = nc.dram_tensor("wis", [CIN, 4 * HID], fp16, kind="ExternalInput")
    wss0_d = nc.dram_tensor("wss0", [HID, 4 * HID], fp16, kind="ExternalInput")
    wss1_d = nc.dram_tensor("wss1", [HID, 4 * HID], fp16, kind="ExternalInput")
    bias_d = nc.dram_tensor("bias", [HID, 4], fp32, kind="ExternalInput")
    out_d = nc.dram_tensor("out", [HID, H * W], fp32, kind="ExternalOutput")

    sig_dt = fp16 if SIG_FP16 else fp32

    with tile.TileContext(nc) as tc:
        with (
            tc.tile_pool(name="persist", bufs=1) as pp,
            tc.tile_pool(name="gates", bufs=3) as gp,
            tc.tile_pool(name="psf", bufs=2, space="PSUM") as psf,
            tc.tile_pool(name="psd", bufs=1, space="PSUM") as psd,
            tc.tile_pool(name="psig", bufs=3, space="PSUM") as psig,
            tc.tile_pool(name="pso", bufs=2, space="PSUM") as pso,
        ):
            xskew = pp.tile([128, T * 64], fp16, tag="xskew")
            wis_s = pp.tile([128, 512], fp16, tag="wis")
            wss0_s = pp.tile([128, 512], fp16, tag="wss0")
            wss1_s = pp.tile([128, 512], fp16, tag="wss1")
            bias_s = pp.tile([128, 4], fp32, tag="bias")
            rhs = [
                pp.tile([128, 66], fp16, tag=f"rhs{i}", name=f"rhs{i}")
                for i in range(2)
            ]
            # cbuf in SBUF; the 8th PSUM bank goes to the pre-filler scratch.
            cbuf = pp.tile([128, 64], fp32, tag="cbuf")
            dumm_ps = psd.tile([128, 96], fp32, tag="dummps")
            scr_s = pp.tile([128, 256], fp32, tag="scrs")
            hist = pp.tile([128, H * W], fp32, tag="hist")
            warm = pp.tile([128, 1], fp32, tag="warm")
            warm2 = pp.tile([128, 64], fp32, tag="warm2")

            # --- prologue ---
            # Spread the gating DMAs across queues so the first z matmul
            # (needs wis + xs chunk0) can start as early as possible: chunk0
            # is small (4 steps) and issued first on the sync queue; wis on
            # scalar; wss on gpsimd (idle until the scan's first hist write);
            # the remaining chunks follow on sync.
            nc.sync.dma_start(out=xskew[:, 0 : 4 * 64], in_=xs_d.ap()[:, 0 : 4 * 64])
            nc.scalar.dma_start(out=wis_s, in_=wis_d.ap())
            nc.gpsimd.dma_start(out=wss0_s, in_=wss0_d.ap())
            nc.gpsimd.dma_start(out=wss1_s, in_=wss1_d.ap())
            nc.scalar.dma_start(out=bias_s, in_=bias_d.ap())
            for k in range(4, 20, 16):
                nc.sync.dma_start(out=xskew[:, k * 64 : 20 * 64], in_=xs_d.ap()[:, k * 64 : 20 * 64])
            for k in range(20, T, 16):
                hi = min(T, k + 16) * 64
                nc.sync.dma_start(out=xskew[:, k * 64 : hi], in_=xs_d.ap()[:, k * 64 : hi])

            # Pull the sigmoid/tanh ACT table load to the start (overlaps DMA).
            nc.vector.memset(warm, 0.0)
            nc.scalar.activation(warm, warm, AFT.Sigmoid)
            nc.scalar.activation(warm, warm, AFT.Tanh)
            nc.vector.memset(warm2, 0.0)

            nc.vector.memset(rhs[0], 0.0)
            nc.vector.memset(rhs[1], 0.0)
            nc.vector.memset(cbuf, 0.0)

            def win(t):
                # active row window: below-diagonal rows are exactly 0 (zero
                # bias) and rows with t-r > 63 are dead. r0 rounded down to
                # even keeps fp16 writes 4B-aligned. Bias path: full width.
                if use_bias:
                    return 0, 63
                r0 = 0 if t < 64 else t - 63
                r1 = t if t < 63 else 63
                return r0 & ~1, r1

            pf = [None] * T
            pig = [None] * T
            po = [None] * T

            def emit_z(t, pin_after=None):
                pf[t] = psf.tile([128, 64], fp32, tag="pf", name=f"pf{t}")
                pig[t] = psig.tile([128, 128], fp32, tag="pig", name=f"pig{t}")
                po[t] = pso.tile([128, 64], fp32, tag="po", name=f"po{t}")
                a, b = win(t)
                r = xskew[:, t * 64 + a : t * 64 + b + 1]
                mf = nc.tensor.matmul(pf[t][:, a : b + 1], lhsT=wis_s[:, 0:128], rhs=r,
                                      start=True, stop=False, skip_group_check=True)
                if pin_after is not None:
                    tile.add_dep_helper(mf.ins, pin_after.ins, sync=False,
                                        reason="z runs right behind rec taps")
                mi = nc.tensor.matmul(pig[t][:, a : b + 1], lhsT=wis_s[:, 128:256], rhs=r,
                                      start=True, stop=False, skip_group_check=True)
                tile.add_dep_helper(mi.ins, mf.ins, sync=False, reason="z b2b chain")
                mg = nc.tensor.matmul(pig[t][:, 64 + a : 64 + b + 1], lhsT=wis_s[:, 256:384], rhs=r,
                                      start=False, stop=False, skip_group_check=True)
                tile.add_dep_helper(mg.ins, mi.ins, sync=False,
                                    reason="bank-clear MM must run first")
                mo = nc.tensor.matmul(po[t][:, a : b + 1], lhsT=wis_s[:, 384:512], rhs=r,
                                      start=True, stop=False, skip_group_check=True)
                tile.add_dep_helper(mo.ins, mg.ins, sync=False, reason="z b2b chain")
                return mo

            emit_z(0)

            prev_tanh = None
            pre_mm = None
            # --- the 127-step scan (gate order: f, i, g, o) ---
            for t in range(T):
                a, b = win(t)
                rbuf = rhs[t % 2]
                tap0 = rbuf[:, 1 + a : 2 + b]
                tap1 = rbuf[:, 2 + a : 3 + b]

                def rec(dst, q, stop, pin_after=None):
                    m0 = nc.tensor.matmul(dst, lhsT=wss0_s[:, q * 128 : (q + 1) * 128], rhs=tap0,
                                          start=False, stop=False, skip_group_check=True)
                    if pin_after is not None:
                        # queue-order pin: the first rec tap issues right
                        # behind the still-streaming pre-filler matmul, so it
                        # gets back-to-back pricing instead of the isolated
                        # pipe-fill.
                        tile.add_dep_helper(m0.ins, pin_after.ins, sync=False,
                                            reason="rec taps queue behind pre-MM filler")
                    return nc.tensor.matmul(dst, lhsT=wss1_s[:, q * 128 : (q + 1) * 128], rhs=tap1,
                                            start=False, stop=stop, skip_group_check=True)

                rec(pf[t][:, a : b + 1], 0, True, pin_after=pre_mm)  # f first
                rec(pig[t][:, a : b + 1], 1, False)           # i
                rec(pig[t][:, 64 + a : 64 + b + 1], 2, True)  # g
                mo_rec = rec(po[t][:, a : b + 1], 3, True)    # o last

                # next step's z_is: ready as soon as the psum banks free up,
                # pinned right behind this step's rec taps so they stream
                # back-to-back into the PE idle window (all off-chain).
                if t + 1 < T:
                    emit_z(t + 1, pin_after=mo_rec)

                sig = gp.tile([128, 192], sig_dt, tag="sig")
                so = gp.tile([128, 64], fp16, tag="so")
                if use_bias:
                    nc.scalar.activation(sig[:, 0:64], pf[t], AFT.Sigmoid, bias=bias_s[:, 0:1])
                    nc.scalar.activation(sig[:, 64:128], pig[t][:, 0:64], AFT.Sigmoid, bias=bias_s[:, 1:2])
                    nc.scalar.activation(sig[:, 128:192], pig[t][:, 64:128], AFT.Sigmoid, bias=bias_s[:, 2:3])
                    nc.scalar.activation(so, po[t], AFT.Sigmoid, bias=bias_s[:, 3:4])
                else:
                    nc.scalar.activation(sig[:, a : b + 1], pf[t][:, a : b + 1], AFT.Sigmoid)
                    nc.scalar.activation(
                        sig[:, 64:192].rearrange("p (g r) -> p g r", g=2)[:, :, a : b + 1],
                        pig[t].rearrange("p (g r) -> p g r", g=2)[:, :, a : b + 1],
                        AFT.Sigmoid,
                    )
                    soi = nc.scalar.activation(so[:, a : b + 1], po[t][:, a : b + 1],
                                               AFT.Sigmoid)

                t1 = gp.tile([128, 64], sig_dt, tag="t1")
                t2 = gp.tile([128, 64], fp32, tag="t2")
                # t2 = sig_f * c ; t1 = (sig_g - 0.5) * sig_i = i*g/2
                nc.vector.tensor_mul(t2[:, a : b + 1], sig[:, a : b + 1], cbuf[:, a : b + 1])
                t1i = nc.vector.scalar_tensor_tensor(
                    t1[:, a : b + 1], sig[:, 128 + a : 128 + b + 1], -0.5,
                    sig[:, 64 + a : 64 + b + 1], ALU.add, ALU.mult
                )
                # c = t1*2 + t2
                nc.vector.scalar_tensor_tensor(
                    cbuf[:, a : b + 1], t1[:, a : b + 1], 2.0, t2[:, a : b + 1],
                    ALU.mult, ALU.add
                )

                tc_s = gp.tile([128, 64], fp16, tag="tc")
                if PRE_TANH and not use_bias:
                    # pre-TANH stretch-filler: runs back-to-back behind sig_o
                    # and spans the ACT idle window until c lands, so the real
                    # TANH (issuable before the filler drains) gets
                    # back-to-back pricing (~253ns vs ~340 isolated).
                    ptw = min(3 * (b - a + 1) + 16, 224)
                    pti = nc.scalar.activation(scr_s[:, 0:ptw],
                                               xskew[:, 0:ptw], AFT.Tanh)
                    tile.add_dep_helper(pti.ins, soi.ins, sync=False,
                                        reason="pre-TANH right behind sig_o")
                tci = nc.scalar.activation(tc_s[:, a : b + 1], cbuf[:, a : b + 1], AFT.Tanh)
                if PRE_TANH and not use_bias:
                    tile.add_dep_helper(tci.ins, pti.ins, sync=False,
                                        reason="TANH right behind its pre-filler")
                prev_tanh = tci

                # pre-MM filler for the NEXT step: starts when this TANH is
                # done and streams through the h hand-off. (Measured: no
                # benefit — the fresh-sem matmul pays its fill regardless —
                # kept behind a flag for reference.)
                if PRE_MM_W and t + 1 < T and not use_bias:
                    pre_mm = nc.tensor.matmul(
                        dumm_ps[:, 0:PRE_MM_W], lhsT=wss0_s[:, 0:128],
                        rhs=xskew[:, 0:PRE_MM_W],
                        start=True, stop=True, skip_group_check=True)
                    tile.add_dep_helper(pre_mm.ins, prev_tanh.ins, sync=True,
                                        reason="pre-MM starts at TANH")
                else:
                    pre_mm = None

                # h (fp16) into the next rhs buffer -- this is the serial chain
                nbuf = rhs[(t + 1) % 2]
                nc.vector.tensor_mul(nbuf[:, 2 + a : 3 + b], so[:, a : b + 1], tc_s[:, a : b + 1])

                # h (fp32) into unskewed history, in-band rows only (off chain)
                r0 = 0 if t < W else t - (W - 1)
                r1 = t if t < W else W - 1
                cnt = r1 - r0 + 1
                base = r0 * 63 + t
                hview = (
                    hist[:, base : base + (cnt - 1) * 63 + 1 : 63]
                    if cnt > 1
                    else hist[:, base : base + 1]
                )
                hist_eng = nc.gpsimd if GP_HIST else nc.vector
                hist_eng.tensor_mul(hview, so[:, r0 : r0 + cnt], tc_s[:, r0 : r0 + cnt])

                # epilogue overlap: rows [k0, k1) are final after step k1-1+63
                for k0, k1 in ((0, 16), (16, 32), (32, 48), (48, 56), (56, 60), (60, 64)):
                    if t == k1 - 1 + 63:
                        nc.sync.dma_start(
                            out=out_d.ap()[:, k0 * 64 : k1 * 64],
                            in_=hist[:, k0 * 64 : k1 * 64],
                        )

    nc.compile()
    return nc


def _get_program(use_bias: bool):
    if use_bias not in _PROGRAM_CACHE:
        _PROGRAM_CACHE[use_bias] = _build_program(use_bias)
    return _PROGRAM_CACHE[use_bias]


def _prep_weights(w):
    """(512, 128) -> (128, 512) fp16 with gate column order [f, i, 2g, o]."""
    wt = w.T.astype(np.float32)  # (128, 512) in [i, f, o, g] order
    out = np.concatenate(
        [wt[:, 128:256], wt[:, 0:128], 2.0 * wt[:, 384:512], wt[:, 256:384]], axis=1
    )
    return np.ascontiguousarray(out.astype(np.float16))


def kernel(x, w_is, b_is, w_ss, b_ss, _trace=False, _trace_kwargs=None):
    from concourse.bass_utils import run_bass_kernel_spmd

    x = np.asarray(x, dtype=np.float32)
    w_is = np.asarray(w_is, dtype=np.float32)
    b_is = np.asarray(b_is, dtype=np.float32)
    w_ss = np.asarray(w_ss, dtype=np.float32)
    b_ss = np.asarray(b_ss, dtype=np.float32)
    B = x.shape[0]
    assert x.shape == (B, CIN, H, W), x.shape

    bias = (b_is + b_ss).astype(np.float32)  # (512,) in [i, f, o, g] order
    use_bias = bool(np.any(bias != 0.0))
    nc = _get_program(use_bias)

    wis_h = _prep_weights(w_is)
    wss0_h = _prep_weights(w_ss[:, :, 0, 0])
    wss1_h = _prep_weights(w_ss[:, :, 1, 0])
    bq = bias.reshape(4, HID)  # [i, f, o, g]
    bias_h = np.ascontiguousarray(
        np.stack([bq[1], bq[0], 2.0 * bq[3], bq[2]], axis=1).astype(np.float32)
    )  # (128, 4) in [f, i, 2g, o] order

    # host-side skew + fp16 cast, t-major: xs[b, c, t*64 + r] = x[b, c, r, t-r]
    xs_all = np.zeros((B, CIN, T, 64), np.float16)
    x16 = x.astype(np.float16)
    for r in range(H):
        xs_all[:, :, r : r + W, r] = x16[:, :, r, :].transpose(0, 1, 2)
    xs_all = xs_all.reshape(B, CIN, T * 64)

    in_maps = []
    for b in range(N_CORES):
        in_maps.append(
            {
                "xs": np.ascontiguousarray(xs_all[b % B]),
                "wis": wis_h,
                "wss0": wss0_h,
                "wss1": wss1_h,
                "bias": bias_h,
            }
        )

    res = run_bass_kernel_spmd(
        nc,
        in_maps,
        core_ids=list(range(N_CORES)),
        trace=_trace,
        **(_trace_kwargs or {}),
    )
    out = np.stack(
        [res.results[b]["out"].reshape(HID, H, W) for b in range(B)], axis=0
    ).astype(np.float32)
    if _trace:
        return out, res
    return out


# revision 30
# speedup vs baseline: 1.1038x; 1.0015x over previous
"""DiagonalLSTM Trainium2 kernel.

Reference computation (per batch element b):
  xs = skew(x)                               # (Cin, H, 2W-1), row r shifted right by r
  z_is = w_is @ xs + b_is                    # 1x1 conv -> 4*HID channels
  for t in 0..2W-2:                          # sequential scan over skewed width
      hs[o, r] = wss[o,c,0] h[c,r-1] + wss[o,c,1] h[c,r] + b_ss[o]
      z = z_is[:, :, t] + hs
      i, f, o_, g = sig, sig, sig, tanh of the 4 gate quarters
      c = f*c + i*g ; h = o_*tanh(c)
  out = unskew(h history)

Sharding: data-parallel over batch B=8 across the 8 NeuronCores (the t-scan is
inherently sequential; each core runs its own batch element's full scan).

Per-core layout (128 partitions = channels), gate order [f, i, g, o]:
 - gates-on-partitions: per step the gates live in THREE psum banks -- f
   (128x64), i|g (128x128), o (128x64).
 - g is computed VIA SIGMOID: tanh(z) = 2*sigmoid(2z) - 1, the factor 2 folded
   into the g columns of all weights host-side.
 - Measured critical chain per step (HW trace, 2072ns at baseline):
     MM f-taps -> sig_f -> sig_ig -> t1 -> c -> TANH -> h-write
   Optimizations in this version, all aimed at that chain:
   * z_is(t+1) matmuls are DELAYED (dep on TANH(t-1)) so the PE streams them
     right up to the moment h(t-1) lands -> the first rec tap issues
     back-to-back instead of paying the isolated-matmul pipe-fill (~210ns).
   * an ACT dummy op after each TANH keeps the scalar engine's pipeline busy
     through the MM segment so sig_f starts back-to-back (saves the ~130-cycle
     restart bubble: 313ns -> ~200ns).
   * sig_o carries a dep on t1 so it issues late and TANH then runs
     back-to-back behind it on the ACT queue (314ns -> ~200ns).
   * sigmoid outputs are fp16 -> t1 runs in the DVE 2x mode.
   * t1 runs on the (otherwise idle) GPSIMD engine, shortening the VE serial
     segment to t2 -> c.
 - x is pre-skewed and pre-cast to fp16 ON THE HOST, t-major (zero padded);
   the step-t rhs is the contiguous slice xs[:, t*64:(t+1)*64].
 - h is written as fp16 into a (128 x 66) rhs buffer with cols 0:2 always 0;
   tap0 (h[r-1]) = cols 1:65 and tap1 (h[r]) = cols 2:66.
 - h history is stored fp32 directly in unskewed layout hist[c, r*64+w] via a
   stride-63 write of the in-band rows; the output DMA is chunked by row
   groups overlapping the scan tail.
 - zero-bias fast path: every per-step op covers only the ACTIVE row window
   [max(0,t-63) & ~1, min(t,63)].
"""

import sys

if "/opt/trn_rl_repo" not in sys.path:
    sys.path.insert(0, "/opt/trn_rl_repo")

import numpy as np

N_CORES = 8
HID = 128
CIN = 128
H = 64
W = 64
T = 2 * W - 1  # 127
LOOKAHEAD = 1
RCHUNK = 16  # epilogue row-chunk size

# --- scan-chain tunables ---
# Measured law (HW trace): a chain instruction waiting on fresh data always
# pays its full isolated-entry cost (pipe fill + access restart); only an op
# that was ISSUABLE before its engine went idle gets back-to-back pricing.
# So each isolated chain entry gets a PRE-FILLER: a scratch op on the same
# engine, dep-pinned to start one chain-unit early and sized to still be
# running when the real op's data lands.
PRE_MM_W = 0    # pre-matmul filler width (0 disables; measured useless — the
                # PE pays its ~200ns fill after a fresh sem regardless)
PRE_TANH = True  # stretch-filler on ACT so the real TANH prices back-to-back
GP_HIST = True  # write the h history on gpsimd instead of vector
SIG_FP16 = False  # stt has no 2x fp16 mode; fp32 sig is no slower

_PROGRAM_CACHE = {}


def _build_program(use_bias: bool):
    import concourse.bacc as bacc
    import concourse.tile as tile
    from concourse import mybir

    fp32 = mybir.dt.float32
    fp16 = mybir.dt.float16
    AFT = mybir.ActivationFunctionType
    ALU = mybir.AluOpType

    nc = bacc.Bacc("TRN2", debug=False, num_devices=N_CORES)
    xs_d = nc.dram_tensor("xs", [CIN, T * 64], fp16, kind="ExternalInput")
    wis_d = nc.dram_tensor("wis", [CIN, 4 * HID], fp16, kind="ExternalInput")
    wss0_d = nc.dram_tensor("wss0", [HID, 4 * HID], fp16, kind="ExternalInput")
    wss1_d = nc.dram_tensor("wss1", [HID, 4 * HID], fp16, kind="ExternalInput")
    bias_d = nc.dram_tensor("bias", [HID, 4], fp32, kind="ExternalInput")
    out_d = nc.dram_tensor("out", [HID, H * W], fp32, kind="ExternalOutput")

    sig_dt = fp16 if SIG_FP16 else fp32

    with tile.TileContext(nc) as tc:
        with (
            tc.tile_pool(name="persist", bufs=1) as pp,
            tc.tile_pool(name="gates", bufs=3) as gp,
            tc.tile_pool(name="psf", bufs=2, space="PSUM") as psf,
            tc.tile_pool(name="psd", bufs=1, space="PSUM") as psd,
            tc.tile_pool(name="psig", bufs=3, space="PSUM") as psig,
            tc.tile_pool(name="pso", bufs=2, space="PSUM") as pso,
        ):
            xskew = pp.tile([128, T * 64], fp16, tag="xskew")
            wis_s = pp.tile([128, 512], fp16, tag="wis")
            wss0_s = pp.tile([128, 512], fp16, tag="wss0")
            wss1_s = pp.tile([128, 512], fp16, tag="wss1")
            bias_s = pp.tile([128, 4], fp32, tag="bias")
            rhs = [
                pp.tile([128, 66], fp16, tag=f"rhs{i}", name=f"rhs{i}")
                for i in range(2)
            ]
            # cbuf in SBUF; the 8th PSUM bank goes to the pre-filler scratch.
            cbuf = pp.tile([128, 64], fp32, tag="cbuf")
            dumm_ps = psd.tile([128, 500], fp32, tag="dummps")
            scr_s = pp.tile([128, 512], fp32, tag="scrs")
            hist = pp.tile([128, H * W], fp32, tag="hist")
            warm = pp.tile([128, 1], fp32, tag="warm")
            warm2 = pp.tile([128, 64], fp32, tag="warm2")

            # --- prologue ---
            # Spread the gating DMAs across queues so the first z matmul
            # (needs wis + xs chunk0) can start as early as possible: chunk0
            # is small (4 steps) and issued first on the sync queue; wis on
            # scalar; wss on gpsimd (idle until the scan's first hist write);
            # the remaining chunks follow on sync.
            nc.sync.dma_start(out=xskew[:, 0 : 4 * 64], in_=xs_d.ap()[:, 0 : 4 * 64])
            nc.scalar.dma_start(out=wis_s, in_=wis_d.ap())
            nc.gpsimd.dma_start(out=wss0_s, in_=wss0_d.ap())
            nc.gpsimd.dma_start(out=wss1_s, in_=wss1_d.ap())
            nc.scalar.dma_start(out=bias_s, in_=bias_d.ap())
            for k in range(4, 20, 16):
                nc.sync.dma_start(out=xskew[:, k * 64 : 20 * 64], in_=xs_d.ap()[:, k * 64 : 20 * 64])
            for k in range(20, T, 16):
                hi = min(T, k + 16) * 64
                nc.sync.dma_start(out=xskew[:, k * 64 : hi], in_=xs_d.ap()[:, k * 64 : hi])

            # Pull the sigmoid/tanh ACT table load to the start (overlaps DMA).
            nc.vector.memset(warm, 0.0)
            nc.scalar.activation(warm, warm, AFT.Sigmoid)
            nc.scalar.activation(warm, warm, AFT.Tanh)
            nc.vector.memset(warm2, 0.0)

            nc.vector.memset(rhs[0], 0.0)
            nc.vector.memset(rhs[1], 0.0)
            nc.vector.memset(cbuf, 0.0)
            nc.vector.memset(dumm_ps, 0.0)

            def win(t):
                # active row window: below-diagonal rows are exactly 0 (zero
                # bias) and rows with t-r > 63 are dead. r0 rounded down to
                # even keeps fp16 writes 4B-aligned. Bias path: full width.
                if use_bias:
                    return 0, 63
                r0 = 0 if t < 64 else t - 63
                r1 = t if t < 63 else 63
                return r0 & ~1, r1

            pf = [None] * T
            pig = [None] * T
            po = [None] * T

            def emit_z(t, pin_after=None):
                pf[t] = psf.tile([128, 64], fp32, tag="pf", name=f"pf{t}")
                pig[t] = psig.tile([128, 128], fp32, tag="pig", name=f"pig{t}")
                po[t] = pso.tile([128, 64], fp32, tag="po", name=f"po{t}")
                a, b = win(t)
                r = xskew[:, t * 64 + a : t * 64 + b + 1]
                mf = nc.tensor.matmul(pf[t][:, a : b + 1], lhsT=wis_s[:, 0:128], rhs=r,
                                      start=True, stop=False, skip_group_check=True)
                if pin_after is not None:
                    tile.add_dep_helper(mf.ins, pin_after.ins, sync=False,
                                        reason="z runs right behind rec taps")
                mi = nc.tensor.matmul(pig[t][:, a : b + 1], lhsT=wis_s[:, 128:256], rhs=r,
                                      start=True, stop=False, skip_group_check=True)
                tile.add_dep_helper(mi.ins, mf.ins, sync=False, reason="z b2b chain")
                mg = nc.tensor.matmul(pig[t][:, 64 + a : 64 + b + 1], lhsT=wis_s[:, 256:384], rhs=r,
                                      start=False, stop=False, skip_group_check=True)
                tile.add_dep_helper(mg.ins, mi.ins, sync=False,
                                    reason="bank-clear MM must run first")
                mo = nc.tensor.matmul(po[t][:, a : b + 1], lhsT=wis_s[:, 384:512], rhs=r,
                                      start=True, stop=False, skip_group_check=True)
                tile.add_dep_helper(mo.ins, mg.ins, sync=False, reason="z b2b chain")
                return mo

            emit_z(0)

            prev_tanh = None
            pre_mm = None
            # --- the 127-step scan (gate order: f, i, g, o) ---
            for t in range(T):
                a, b = win(t)
                rbuf = rhs[t % 2]
                tap0 = rbuf[:, 1 + a : 2 + b]
                tap1 = rbuf[:, 2 + a : 3 + b]

                def rec(dst, q, stop, pin_after=None):
                    m0 = nc.tensor.matmul(dst, lhsT=wss0_s[:, q * 128 : (q + 1) * 128], rhs=tap0,
                                          start=False, stop=False, skip_group_check=True)
                    if pin_after is not None:
                        # queue-order pin: the first rec tap issues right
                        # behind the still-streaming pre-filler matmul, so it
                        # gets back-to-back pricing instead of the isolated
                        # pipe-fill.
                        tile.add_dep_helper(m0.ins, pin_after.ins, sync=False,
                                            reason="rec taps queue behind pre-MM filler")
                    return nc.tensor.matmul(dst, lhsT=wss1_s[:, q * 128 : (q + 1) * 128], rhs=tap1,
                                            start=False, stop=stop, skip_group_check=True)

                rec(pf[t][:, a : b + 1], 0, True, pin_after=pre_mm)  # f first
                rec(pig[t][:, a : b + 1], 1, False)           # i
                rec(pig[t][:, 64 + a : 64 + b + 1], 2, True)  # g
                mo_rec = rec(po[t][:, a : b + 1], 3, True)    # o last

                # next step's z_is: ready as soon as the psum banks free up,
                # pinned right behind this step's rec taps so they stream
                # back-to-back into the PE idle window (all off-chain).
                if t + 1 < T:
                    emit_z(t + 1, pin_after=mo_rec)

                # pre-sigmoid_f stretch-filler: occupies ACT from TANH(t-1)
                # through the matmul segment so sig_f prices back-to-back.
                # Reads the scratch PSUM bank (same source space as sig_f).
                if prev_tanh is not None and not use_bias:
                    psw = min(7 * (b - a + 1) + 88, 500)
                    psf_i = nc.scalar.activation(scr_s[:, 0:psw],
                                                 dumm_ps[:, 0:psw], AFT.Sigmoid)
                    tile.add_dep_helper(psf_i.ins, prev_tanh.ins, sync=False,
                                        reason="pre-sig_f right behind TANH(t-1)")

                sig = gp.tile([128, 192], sig_dt, tag="sig")
                so = gp.tile([128, 64], fp16, tag="so")
                if use_bias:
                    nc.scalar.activation(sig[:, 0:64], pf[t], AFT.Sigmoid, bias=bias_s[:, 0:1])
                    nc.scalar.activation(sig[:, 64:128], pig[t][:, 0:64], AFT.Sigmoid, bias=bias_s[:, 1:2])
                    nc.scalar.activation(sig[:, 128:192], pig[t][:, 64:128], AFT.Sigmoid, bias=bias_s[:, 2:3])
                    nc.scalar.activation(so, po[t], AFT.Sigmoid, bias=bias_s[:, 3:4])
                else:
                    sfi = nc.scalar.activation(sig[:, a : b + 1], pf[t][:, a : b + 1], AFT.Sigmoid)
                    if prev_tanh is not None:
                        tile.add_dep_helper(sfi.ins, psf_i.ins, sync=False,
                                            reason="sig_f right behind its pre-filler")
                    nc.scalar.activation(
                        sig[:, 64:192].rearrange("p (g r) -> p g r", g=2)[:, :, a : b + 1],
                        pig[t].rearrange("p (g r) -> p g r", g=2)[:, :, a : b + 1],
                        AFT.Sigmoid,
                    )
                    soi = nc.scalar.activation(so[:, a : b + 1], po[t][:, a : b + 1],
                                               AFT.Sigmoid)

                t1 = gp.tile([128, 64], sig_dt, tag="t1")
                t2 = gp.tile([128, 64], fp32, tag="t2")
                # t2 = sig_f * c ; t1 = (sig_g - 0.5) * sig_i = i*g/2
                nc.vector.tensor_mul(t2[:, a : b + 1], sig[:, a : b + 1], cbuf[:, a : b + 1])
                t1i = nc.vector.scalar_tensor_tensor(
                    t1[:, a : b + 1], sig[:, 128 + a : 128 + b + 1], -0.5,
                    sig[:, 64 + a : 64 + b + 1], ALU.add, ALU.mult
                )
                # c = t1*2 + t2
                nc.vector.scalar_tensor_tensor(
                    cbuf[:, a : b + 1], t1[:, a : b + 1], 2.0, t2[:, a : b + 1],
                    ALU.mult, ALU.add
                )

                tc_s = gp.tile([128, 64], fp16, tag="tc")
                if PRE_TANH and not use_bias:
                    # pre-TANH stretch-filler: runs back-to-back behind sig_o
                    # and spans the ACT idle window until c lands, so the real
                    # TANH (issuable before the filler drains) gets
                    # back-to-back pricing (~253ns vs ~340 isolated).
                    ptw = min(3 * (b - a + 1) + 64, 256)
                    pti = nc.scalar.activation(scr_s[:, 0:ptw],
                                               xskew[:, 0:ptw], AFT.Tanh)
                    tile.add_dep_helper(pti.ins, soi.ins, sync=False,
                                        reason="pre-TANH right behind sig_o")
                tci = nc.scalar.activation(tc_s[:, a : b + 1], cbuf[:, a : b + 1], AFT.Tanh)
                if PRE_TANH and not use_bias:
                    tile.add_dep_helper(tci.ins, pti.ins, sync=False,
                                        reason="TANH right behind its pre-filler")
                prev_tanh = tci

                # pre-MM filler for the NEXT step: starts when this TANH is
                # done and streams through the h hand-off. (Measured: no
                # benefit — the fresh-sem matmul pays its fill regardless —
                # kept behind a flag for reference.)
                if PRE_MM_W and t + 1 < T and not use_bias:
                    pre_mm = nc.tensor.matmul(
                        dumm_ps[:, 0:PRE_MM_W], lhsT=wss0_s[:, 0:128],
                        rhs=xskew[:, 0:PRE_MM_W],
                        start=True, stop=True, skip_group_check=True)
                    tile.add_dep_helper(pre_mm.ins, prev_tanh.ins, sync=True,
                                        reason="pre-MM starts at TANH")
                else:
                    pre_mm = None

                # h (fp16) into the next rhs buffer -- this is the serial chain
                nbuf = rhs[(t + 1) % 2]
                nc.vector.tensor_mul(nbuf[:, 2 + a : 3 + b], so[:, a : b + 1], tc_s[:, a : b + 1])

                # h (fp32) into unskewed history, in-band rows only (off chain)
                r0 = 0 if t < W else t - (W - 1)
                r1 = t if t < W else W - 1
                cnt = r1 - r0 + 1
                base = r0 * 63 + t
                hview = (
                    hist[:, base : base + (cnt - 1) * 63 + 1 : 63]
                    if cnt > 1
                    else hist[:, base : base + 1]
                )
                hist_eng = nc.gpsimd if GP_HIST else nc.vector
                hist_eng.tensor_mul(hview, so[:, r0 : r0 + cnt], tc_s[:, r0 : r0 + cnt])

                # epilogue overlap: rows [k0, k1) are final after step k1-1+63
                for k0, k1 in ((0, 16), (16, 32), (32, 48), (48, 56), (56, 60),
                               (60, 62), (62, 63), (63, 64)):
                    if t == k1 - 1 + 63:
                        nc.sync.dma_start(
                            out=out_d.ap()[:, k0 * 64 : k1 * 64],
                            in_=hist[:, k0 * 64 : k1 * 64],
                        )

    nc.compile()
    return nc


def _get_program(use_bias: bool):
    if use_bias not in _PROGRAM_CACHE:
        _PROGRAM_CACHE[use_bias] = _build_program(use_bias)
    return _PROGRAM_CACHE[use_bias]


def _prep_weights(w):
    """(512, 128) -> (128, 512) fp16 with gate column order [f, i, 2g, o]."""
    wt = w.T.astype(np.float32)  # (128, 512) in [i, f, o, g] order
    out = np.concatenate(
        [wt[:, 128:256], wt[:, 0:128], 2.0 * wt[:, 384:512], wt[:, 256:384]], axis=1
    )
    return np.ascontiguousarray(out.astype(np.float16))


def kernel(x, w_is, b_is, w_ss, b_ss, _trace=False, _trace_kwargs=None):
    from concourse.bass_utils import run_bass_kernel_spmd

    x = np.asarray(x, dtype=np.float32)
    w_is = np.asarray(w_is, dtype=np.float32)
    b_is = np.asarray(b_is, dtype=np.float32)
    w_ss = np.asarray(w_ss, dtype=np.float32)
    b_ss = np.asarray(b_ss, dtype=np.float32)
    B = x.shape[0]
    assert x.shape == (B, CIN, H, W), x.shape

    bias = (b_is + b_ss).astype(np.float32)  # (512,) in [i, f, o, g] order
    use_bias = bool(np.any(bias != 0.0))
    nc = _get_program(use_bias)

    wis_h = _prep_weights(w_is)
    wss0_h = _prep_weights(w_ss[:, :, 0, 0])
    wss1_h = _prep_weights(w_ss[:, :, 1, 0])
    bq = bias.reshape(4, HID)  # [i, f, o, g]
    bias_h = np.ascontiguousarray(
        np.stack([bq[1], bq[0], 2.0 * bq[3], bq[2]], axis=1).astype(np.float32)
    )  # (128, 4) in [f, i, 2g, o] order

    # host-side skew + fp16 cast, t-major: xs[b, c, t*64 + r] = x[b, c, r, t-r]
    xs_all = np.zeros((B, CIN, T, 64), np.float16)
    x16 = x.astype(np.float16)
    for r in range(H):
        xs_all[:, :, r : r + W, r] = x16[:, :, r, :].transpose(0, 1, 2)
    xs_all = xs_all.reshape(B, CIN, T * 64)

    in_maps = []
    for b in range(N_CORES):
        in_maps.append(
            {
                "xs": np.ascontiguousarray(xs_all[b % B]),
                "wis": wis_h,
                "wss0": wss0_h,
                "wss1": wss1_h,
                "bias": bias_h,
            }
        )

    res = run_bass_kernel_spmd(
        nc,
        in_maps,
        core_ids=list(range(N_CORES)),
        trace=_trace,
        **(_trace_kwargs or {}),
    )
    out = np.stack(
        [res.results[b]["out"].reshape(HID, H, W) for b in range(B)], axis=0
    ).astype(np.float32)
    if _trace:
        return out, res
    return out
